# revision 83
# baseline (speedup 1.0000x reference)
"""Multi-head attention (B=2, S=2048, E=1024, H=16) on 8 Trainium2 NeuronCores.

Sharding: core c -> batch c//4, heads 4*(c%4)..4*(c%4)+3 (data + head
parallel).  Each core emits a bf16 partial output projection [S, E] over its
256 head-dims; the host sums the 4 partials per batch in f32 and adds the
output bias (the "all-reduce" happens in the unshard step).

Numerics (validated off-line against a float64 reference and on-device:
rel err 0.0055 vs the 2e-2 gate; plain-fp8 operands measured 0.025-0.045
and are NOT used):
  * Projections run on the PE in fp8e4m3 DoubleRow mode (0.5 cycles/row,
    2 contraction chunks per instruction) using an exact hi/lo split:
    x @ W ~= x8@W8 + x8@Wr + xr@W8  (xr/Wr are the fp8 residuals; the
    dropped xr@Wr term is ~2^-8 relative).  Host precomputes all splits.
    Weights carry a x16 scale (and Wq the 1/sqrt(D) fold): QT/KT hold 16q,
    2q*... the exp activation's scale=1/256 de-scales the score product, and
    the x16 on V cancels against the x16 ones-column through the softmax
    normalization, so no extra de-scale ops exist anywhere.
  * Scores/PV/out-projection stay bf16 (DoubleRow needs both operands fp8;
    measured fp8 probs/V/attn all bust the error budget).  Score AND PV
    matmuls are column-trimmed to each diagonal key-tile's live span (the
    first PV matmul keeps full width with start=True so every PSUM address
    is initialized; the skipped columns were exact zero probs).
  * Softmax: constant-shift exp (shift=2), masking applied post-exp as a 0/1
    multiply on the diagonal blocks; Z comes from a 16-valued ones column in
    V via the same PV matmuls; zinv = DVE reciprocal -> bf16, broadcast
    across partitions with a K=1 PE matmul.

Schedule (cost-model-driven; container has no NTFF/neuron-profile path;
TimelineSim 136165 ns vs the 178675 ns bf16 baseline):
  * DMA priority order: Q then K streams (hi then lo, weight tile leading
    each), V last; the Q projection (both m-tiles, 3 split passes, e-pair
    outer, per-group PSUM copies emitted the moment each group stops) runs
    while its chunks stream, then K.  Scores for 3 heads of the first
    q-group are emitted as a prelude that fills the xv-stream window (one
    more head mid-V); the V projection is two-stage (hi passes -> f32 acc
    in SBUF, residual pass + DVE add) so its PSUM groups never wait on the
    lo stream.
  * Attention runs over a flattened (group, head) task list -- group order
    [largest, 2nd-smallest, 2nd-largest, smallest]: the prelude then hides
    the largest group's exp chain under the V phase, and the kernel tail
    ends on the smallest group --
    software-pipelined: scores(i+1..i+depth) are emitted before PV(i) so
    every PV's exp wait is covered by the next head's matmuls, and the
    zinv/attn chain (DVE reciprocals staging a head-pair's zinv rows at
    partitions 0/D of a zeroed tile -> one K=D+1 selector matmul broadcasts
    both -> DVE muls of the SBUF ev copies with the PSUM broadcast;
    hardware allows only one PSUM operand per vector op) trails a pair
    behind.  The staging tile MUST be memset first: its zero-weight rows
    still enter the contraction and uninitialized SBUF NaNs poison it on
    real hardware (the simulator runs no numerics).
  * The previous group's output projection is sprinkled one PSUM-group at
    a time between the scores/PV/fin stages so the 2-deep psO ring (shared
    with the zinv broadcast) never stalls the PE; the last two groups'
    second e-half copies go to ACT to unload the DVE in the tail, and the
    final group's output DMAs go per e-half so the epilogue's last transfer
    starts right after its copy instead of behind the whole-stile chain.
    The last two groups' ev copies also run on ACT: in the tail the DVE
    queue (muls + out-copies) is the local bottleneck while exp is small.
PSUM budget: psS 2x[P,2,QG] (4 banks) + psPV 2x[D+1,QG] (2) + psO/bcast
2x[P,QG] (2) = 8 banks exactly.
"""

import sys

for _p in ("/opt/trn_rl_repo", "/root/.axon_site/_ro/trn_rl_repo"):
    if _p not in sys.path:
        sys.path.insert(0, _p)

import numpy as np


# ---------------------------------------------------------------------------
# Patch: the walrus build in this container rejects >1 sem wait on one CTRL
# instruction ("Too many sync wait commands") and the TileContext exit drain
# aggregates every outstanding proc's wait onto a single Drain. Spill the
# excess waits onto SP nops (1 wait each) emitted right after the drain.
# ---------------------------------------------------------------------------
def _install_tile_drain_patch():
    import concourse.tile as tile
    import concourse.mybir as mybir
    from concourse.vector_clock import ScopedClock

    if getattr(tile.TileContext, "_drain_patch_installed", False):
        return

    def _patched_drain_and_barrier(self, tick_clock, wait_clock):
        drain_inst = self.nc.sync.drain()
        wait_clock.add_sem_waits(
            drain_inst.ins, ScopedClock({None: tick_clock.global_clock})
        )
        si = drain_inst.ins.sync_info
        waits = list(si.on_wait) if si and si.on_wait else []
        if len(waits) > 1:
            si.on_wait = waits[:1]
            for w in waits[1:]:
                nop = self.nc.sync.nop(nofuse=True, hint="drain_wait_spill")
                nop.ins.sync_info = mybir.SyncInfo(on_wait=[w], on_update=[])
        self.nc.all_engine_barrier()
        assert self.sems is not None
        popped = self.nc._tile_sem_poison_stack.pop()
        assert popped is self._sem_poison
        self.nc.clear_and_free_semaphores(list(self.sems.allocated().values()))
        self.nc.all_engine_barrier()

    tile.TileContext._drain_and_barrier = _patched_drain_and_barrier
    tile.TileContext._drain_patch_installed = True


def _split_multi_waits(nc, maxw=1):
    """Walrus here allows only `maxw` sem-wait commands per instruction.
    Hoist excess waits onto engine-queue NoOps inserted just before the
    instruction (the sequencer executes them in order, so semantics are
    identical)."""
    import concourse.mybir as mybir

    ctr = 0
    for bb in nc.main_func.blocks:
        new = []
        for inst in bb.instructions:
            si = inst.sync_info
            waits = list(si.on_wait) if si and si.on_wait else []
            if len(waits) > maxw:
                extras = waits[:-maxw]
                si.on_wait = waits[-maxw:]
                for i in range(0, len(extras), maxw):
                    nop = mybir.InstNoOp(
                        name=f"I-waitspill-{ctr}", engine=inst.engine,
                        ins=[], outs=[])
                    ctr += 1
                    nop.sync_info = mybir.SyncInfo(
                        on_wait=extras[i:i + maxw], on_update=[])
                    try:
                        nc.register_instruction(nop, overwrite=True)
                    except Exception:
                        pass
                    new.append(nop)
            new.append(inst)
        bb.instructions = new


# ---------------------------------------------------------------------------
# Mask classification (host side, from the actual mask array).
# Blocks are 128x128 in the *transposed* score layout: block (kt, qb) covers
# keys kt*128.. x queries qb*128... Returns per-block bias indices into a
# stack of unique multiplicative 0/1 mask blocks.
# ---------------------------------------------------------------------------
def classify_mask(mask2d, S, KB=128):
    nb = S // KB
    assert mask2d.shape == (S, S)
    assert mask2d.any(axis=1).all(), "a query row with no attended key"
    maskT = mask2d.T  # [keys, q]
    uniq = {}
    biases = []
    bias_idx = {}  # (kt, qb) -> None (all attended) or index
    block_live = np.zeros((nb, nb), dtype=bool)  # any attended key in block
    for kt in range(nb):
        for qb in range(nb):
            blk = maskT[kt * KB:(kt + 1) * KB, qb * KB:(qb + 1) * KB]
            if blk.all():
                bias_idx[(kt, qb)] = None
                block_live[kt, qb] = True
            else:
                b = np.where(blk, np.float32(1.0), np.float32(0.0))
                key = b.tobytes()
                if key not in uniq:
                    uniq[key] = len(biases)
                    biases.append(b)
                bias_idx[(kt, qb)] = uniq[key]
                block_live[kt, qb] = blk.any()
    return bias_idx, biases, block_live


# ---------------------------------------------------------------------------
# Bass program builder (one SPMD program, same for all cores).
# ---------------------------------------------------------------------------
def build_nc(S, E, D, HL, bias_idx, block_live, nuniq, shift=2.0, repeat=1):
    import concourse.bass as bass
    import concourse.mybir as mybir
    import concourse.tile as tile

    f32 = mybir.dt.float32
    bf16 = mybir.dt.bfloat16
    f8 = mybir.dt.float8e4
    Act = mybir.ActivationFunctionType
    DR = mybir.MatmulPerfMode.DoubleRow
    ADD = mybir.AluOpType.add

    P = 128
    EC2 = E // 256           # contraction chunk-pairs (4)
    DIM = HL * D             # this core's head dims (256)
    MT = DIM // P            # m-tiles of QT/KT (2)
    QG = 512                 # q-group width
    NQG = S // QG
    QB = QG // P             # q-blocks per group
    NKT = S // P             # key tiles
    NST = S // P
    VW = HL * (D + 1)        # V width incl. ones columns (260)
    SCINV = 1.0 / 256.0      # undo the 2x/16x weight scales in the exp
    VONES = 16.0             # ones-column value matching the 16x on Wv
    N_PRELUDE = 3            # heads of the first q-group emitted early

    def kts_for_group(g):
        out = []
        for kt in range(NKT):
            if any(block_live[kt, g * QB + j] for j in range(QB)):
                out.append(kt)
        return out

    def span_start(kt, g):
        js = [j for j in range(QB) if block_live[kt, g * QB + j]]
        return min(js) * P

    nc = bass.Bass()
    dp = nc.declare_dram_parameter
    d_x = {}
    for t in ("q", "k", "v"):
        for sfx in ("8", "r"):
            d_x[t + sfx] = dp(f"x{t}{sfx}", [E, S], f8, isOutput=False)
    WWID = {"q": DIM, "k": DIM, "v": VW}
    d_w = {}
    for t, wid in WWID.items():
        for sfx in ("8", "r"):
            d_w[t + sfx] = dp(f"w{t}{sfx}", [P, EC2, 2, wid], f8,
                              isOutput=False)
    d_wo = dp("wo", [DIM, E], bf16, isOutput=False)
    d_bias = dp("biasT", [P, max(nuniq, 1) * P], bf16, isOutput=False)
    d_out = dp("out_p", [S, E], bf16, isOutput=True)

    import contextlib
    with tile.TileContext(nc) as tc, contextlib.ExitStack() as _stk:
        consts = _stk.enter_context(tc.tile_pool(name="consts", bufs=1))

        w_sb = {}
        for t, wid in WWID.items():
            for sfx in ("8", "r"):
                k = t + sfx
                w_sb[k] = consts.tile([P, EC2, 2, wid], f8, name=f"sb_w{k}",
                                      tag=f"sb_w{k}")
        wo_sb = [consts.tile([2 * D, E], bf16, name=f"sb_wo{p}",
                             tag=f"sb_wo{p}") for p in range(HL // 2)]
        bias_sb = consts.tile([P, max(nuniq, 1) * P], bf16, name="sb_bias")
        ones128 = consts.tile([P, D], bf16, name="ones128")
        nc.vector.memset(ones128, 1.0)
        # pair-broadcast selector: routes z2 row 0 -> out rows 0..D-1 and
        # z2 row D -> out rows D..2D-1 in a single K=D+1 matmul
        ones2 = consts.tile([D + 1, 2 * D], bf16, name="ones2")
        nc.vector.memset(ones2, 0.0)
        nc.vector.memset(ones2[0:1, 0:D], 1.0)
        nc.vector.memset(ones2[D:D + 1, D:2 * D], 1.0)
        negshift = consts.tile([P, 1], f32, name="negshift")
        nc.vector.memset(negshift, -shift)

        def emit_once():
            QT = [consts.tile([P, S], bf16, name=f"QT{m}", tag=f"QT{m}")
                  for m in range(MT)]
            KT = [consts.tile([P, S], bf16, name=f"KT{m}", tag=f"KT{m}")
                  for m in range(MT)]
            V = [consts.tile([P, VW], bf16, name=f"V{s}", tag=f"V{s}")
                 for s in range(NST)]
            attnP = [[consts.tile([2 * D, QG], bf16, name=f"attnP{p}g{g}",
                                  tag=f"attnP{p}g{g}") for g in range(NQG)]
                     for p in range(HL // 2)]

            # ---------------- DMA emission (order = priority) -------------
            # pool alloc/release must be LIFO per memory space
            xv_pool = tc.alloc_tile_pool(name="xvp", bufs=1)
            xqk_pool = tc.alloc_tile_pool(name="xqk", bufs=1)

            x_sb = {}

            def load_x(pool, key, ep, halves=1):
                t = pool.tile([P, 2, S], f8, name=f"x{key}e{ep}",
                              tag=f"x{key}e{ep}")
                src_ap = d_x[key][ep * 256:(ep + 1) * 256, :].rearrange(
                    "(t p) s -> p t s", p=P)
                hw = S // halves
                for i in range(halves):
                    nc.sync.dma_start(out=t[:, :, i * hw:(i + 1) * hw],
                                      in_=src_ap[:, :, i * hw:(i + 1) * hw])
                x_sb.setdefault(key, {})[ep] = t

            def load_w(key, split=False):
                if split:
                    # per-chunk-pair slices so the first projection matmul
                    # only waits for a 64KB transfer, not the whole tile
                    for ep in range(EC2):
                        nc.sync.dma_start(out=w_sb[key][:, ep, :, :],
                                          in_=d_w[key][:, ep, :, :])
                else:
                    nc.sync.dma_start(out=w_sb[key], in_=d_w[key][:, :, :, :])

            # each stream leads with its (small) weight tile so the first
            # chunk's matmul has both operands as early as possible
            for key, pool in (("q8", xqk_pool), ("qr", xqk_pool),
                              ("k8", xqk_pool), ("kr", xqk_pool),
                              ("v8", xv_pool), ("vr", xv_pool)):
                load_w(key)
                for ep in range(EC2):
                    load_x(pool, key, ep)
            for p in range(HL // 2):
                nc.sync.dma_start(
                    out=wo_sb[p], in_=d_wo[p * 2 * D:(p + 1) * 2 * D, :])
            nc.sync.dma_start(out=bias_sb, in_=d_bias[:, :])

            PASSES = (("8", "8"), ("8", "r"), ("r", "8"))

            # ---------------- Q then K projections (fp8 split) ------------
            psQK = tc.alloc_tile_pool(name="psQK", bufs=1, space="PSUM")

            def emit_proj(proj, dst, copy_eng):
                ps = {}
                for m in range(MT):
                    for g in range(NQG):
                        ps[m, g] = psQK.tile([P, QG], f32,
                                             tag=f"psqk{m}{g}",
                                             name=f"ps{proj}{m}{g}")
                def copy_out(m, g):
                    # engines split per m-tile so the m0 tiles (which gate
                    # the prelude scores) drain in parallel with m1
                    eng = copy_eng[m]
                    dstap = dst[m][:, g * QG:(g + 1) * QG]
                    if eng == "act":
                        nc.scalar.copy(dstap, ps[m, g])
                    elif eng == "pool":
                        nc.gpsimd.tensor_copy(dstap, ps[m, g])
                    else:
                        nc.vector.tensor_copy(dstap, ps[m, g])

                for pi, (xs, ws) in enumerate(PASSES):
                    for ep in range(EC2):
                        last = pi == 2 and ep == EC2 - 1
                        for m in range(MT):
                            for g in range(NQG):
                                nc.tensor.matmul(
                                    ps[m, g],
                                    lhsT=w_sb[proj + ws][
                                        :, ep, :, m * P:(m + 1) * P],
                                    rhs=x_sb[proj + xs][ep][
                                        :, :, g * QG:(g + 1) * QG],
                                    start=(pi == 0 and ep == 0),
                                    stop=last,
                                    perf_mode=DR)
                                if last:
                                    # copy each group the moment it stops, so
                                    # the first scores don't wait for all 8
                                    copy_out(m, g)

            emit_proj("q", QT, {0: "dve", 1: "dve"})
            emit_proj("k", KT, {0: "act", 1: "dve"})
            psQK.release()
            xqk_pool.release()

            # ---------------- phase-B pools -------------------------------
            psS = tc.alloc_tile_pool(name="psS", bufs=2, space="PSUM")
            probs_pool = tc.alloc_tile_pool(name="probs", bufs=34)
            z_pool = tc.alloc_tile_pool(name="zrb", bufs=4)
            out_pool = tc.alloc_tile_pool(name="outst", bufs=4)

            def emit_probs(h, g):
                m, po = h // 2, (h % 2) * D
                kts = kts_for_group(g)
                pairs = [kts[i:i + 2] for i in range(0, len(kts), 2)]
                plist = []
                for pair in pairs:
                    sps = psS.tile([P, 2, QG], f32, tag="s")
                    pb = probs_pool.tile([P, 2, QG], bf16, tag="pb")
                    spans = [span_start(kt, g) for kt in pair]
                    for x, kt in enumerate(pair):
                        s0 = spans[x]
                        nc.tensor.matmul(
                            sps[:, x, s0:],
                            lhsT=KT[m][po:po + D, kt * P:(kt + 1) * P],
                            rhs=QT[m][po:po + D, g * QG + s0:(g + 1) * QG],
                            start=True, stop=True)
                    if len(pair) == 2:
                        # one op from the smaller span: any dead columns it
                        # covers are re-zeroed by the masking memsets below
                        s0 = min(spans)
                        nc.scalar.activation(
                            pb[:, :, s0:], sps[:, :, s0:],
                            Act.Exp, bias=negshift[:, 0:1], scale=SCINV)
                    else:
                        nc.scalar.activation(
                            pb[:, 0, spans[0]:], sps[:, 0, spans[0]:],
                            Act.Exp, bias=negshift[:, 0:1], scale=SCINV)
                    for x, kt in enumerate(pair):
                        s0 = spans[x]
                        for j in range(QB):
                            qb = g * QB + j
                            if j * P < s0:
                                continue
                            bidx = bias_idx[(kt, qb)]
                            if bidx is None:
                                continue
                            blk = pb[:, x, j * P:(j + 1) * P]
                            if not block_live[kt, qb]:
                                nc.gpsimd.memset(blk, 0.0)
                            else:
                                nc.vector.tensor_mul(
                                    blk, blk,
                                    bias_sb[:, bidx * P:(bidx + 1) * P])
                    plist.append((pair, pb, spans))
                return plist

            by_size = sorted(range(NQG), key=lambda g: -len(kts_for_group(g)))
            g_order = ([by_size[0], by_size[2], by_size[1], by_size[3]]
                       if NQG > 3 else by_size)

            # prelude: scores for the first heads of the largest q-group fill
            # the xv-stream window
            probs_cache = {}
            for h in range(N_PRELUDE):
                probs_cache[(h, g_order[0])] = emit_probs(h, g_order[0])

            # ---------------- V projection (two-stage fp8 split) ----------
            psV = tc.alloc_tile_pool(name="psV", bufs=1, space="PSUM")
            accv_pool = tc.alloc_tile_pool(name="accv", bufs=1)
            accV = [accv_pool.tile([P, VW], f32, tag=f"accv{st}",
                                   name=f"accv{st}") for st in range(NST)]
            for quarter in range(NST // 4):
                sts = range(quarter * 4, quarter * 4 + 4)
                pss = [psV.tile([P, VW], f32, tag=f"psv{i}", name=f"psv{i}")
                       for i in range(4)]
                for pi, (xs, ws) in enumerate(PASSES[:2]):
                    for ep in range(EC2):
                        for i, st in enumerate(sts):
                            nc.tensor.matmul(
                                pss[i],
                                lhsT=x_sb["v" + xs][ep][
                                    :, :, st * P:(st + 1) * P],
                                rhs=w_sb["v" + ws][:, ep, :, :],
                                start=(pi == 0 and ep == 0),
                                stop=(pi == 1 and ep == EC2 - 1),
                                perf_mode=DR)
                for i, st in enumerate(sts):
                    nc.vector.tensor_copy(accV[st], pss[i])
                if quarter == 0 and N_PRELUDE < HL:
                    # one more head of early scores fills the xv-stream wait
                    probs_cache[(N_PRELUDE, g_order[0])] = emit_probs(
                        N_PRELUDE, g_order[0])
            for quarter in range(NST // 4):
                sts = range(quarter * 4, quarter * 4 + 4)
                pss = [psV.tile([P, VW], f32, tag=f"psv{i}", name=f"psvr{i}")
                       for i in range(4)]
                for ep in range(EC2):
                    for i, st in enumerate(sts):
                        nc.tensor.matmul(
                            pss[i],
                            lhsT=x_sb["vr"][ep][:, :, st * P:(st + 1) * P],
                            rhs=w_sb["v8"][:, ep, :, :],
                            start=(ep == 0), stop=(ep == EC2 - 1),
                            perf_mode=DR)
                for i, st in enumerate(sts):
                    nc.vector.tensor_tensor(V[st], pss[i], accV[st], ADD)
                    onescols = V[st].rearrange(
                        "p (h c) -> p h c", c=D + 1)[:, :, D]
                    nc.gpsimd.memset(onescols, VONES)
            accv_pool.release()
            psV.release()

            # ---------------- attention main loop -------------------------
            psPV = tc.alloc_tile_pool(name="psPV", bufs=2, space="PSUM")
            # the zinv-broadcast psum and the out-projection psum share one
            # 2-deep [P, QG] ring: the broadcast only borrows it 16 times,
            # and outproj pieces get double-buffering so the PE never waits
            # for the previous piece's PSUM->SBUF copy
            psO = tc.alloc_tile_pool(name="psO", bufs=2, space="PSUM")

            def emit_pv(h, g, plist):
                kts = kts_for_group(g)
                pv = psPV.tile([D + 1, QG], f32, tag="pv")
                n = 0
                for pair, pb, spans in plist:
                    for x, kt in enumerate(pair):
                        s0 = 0 if n == 0 else spans[x]
                        nc.tensor.matmul(
                            pv[:, s0:],
                            lhsT=V[kt][:, h * (D + 1):(h + 1) * (D + 1)],
                            rhs=pb[:, x, s0:],
                            start=(n == 0), stop=(n == len(kts) - 1))
                        n += 1
                # zinv rows of the two heads of a pair stage at partitions
                # 0 and D of a shared tile for a single pair-broadcast matmul
                if h % 2 == 0:
                    z2 = z_pool.tile([D + 1, QG], bf16, tag="z2")
                    # the unwritten rows take part in the selector
                    # contraction with zero weights; clear them so they
                    # cannot hold NaN/Inf garbage (the recips then overwrite
                    # rows 0 and D)
                    nc.gpsimd.memset(z2, 0.0)
                    pair_z2[(g, h // 2)] = z2
                else:
                    z2 = pair_z2[(g, h // 2)]
                with nc.allow_low_precision("zinv broadcast is bf16, as was "
                                            "the baseline's exp(-ln Z)"):
                    nc.vector.reciprocal(z2[(h % 2) * D:(h % 2) * D + 1, :],
                                         pv[D:D + 1, :])
                ev = z_pool.tile([D, QG], f32, tag="ev")
                if g in g_order[-2:]:
                    # tail groups: the DVE queue is locally saturated with
                    # muls and out-copies while exp work is small -> ACT
                    nc.scalar.copy(ev, pv[0:D, :])
                else:
                    nc.vector.tensor_copy(ev, pv[0:D, :])
                return ev

            pair_z2 = {}

            def emit_pairfin(g, p, ev0, ev1):
                # one K=D+1 matmul broadcasts both heads' zinv rows across
                # their 64-partition halves; the muls pair the SBUF ev
                # copies with the PSUM broadcast (hardware allows only one
                # PSUM operand per vector op)
                z2 = pair_z2.pop((g, p))
                obt = psO.tile([P, QG], f32, tag="o")
                nc.tensor.matmul(obt, lhsT=ones2[:, :], rhs=z2[:, :],
                                 start=True, stop=True)
                nc.vector.tensor_mul(attnP[p][g][0:D, :], ev0, obt[0:D, :])
                nc.vector.tensor_mul(attnP[p][g][D:2 * D, :], ev1,
                                     obt[D:2 * D, :])

            def outproj_pieces(g, alt=False, drain=False):
                # one closure per (stile, e-half): a single psO group each,
                # sprinkled into the scores stream so the 2-deep psO ring
                # never stalls the PE; the PSUM->SBUF bounce is on the DVE
                # (GPSIMD cannot read PSUM on hardware)
                pieces = []
                for j in range(QB):
                    st = g * QB + j
                    ot = out_pool.tile([P, E], bf16, tag="ot")

                    def mk(j=j, st=st, ot=ot):
                        def eg_piece(eg):
                            ops = psO.tile([P, QG], f32, tag="o")
                            for p in range(HL // 2):
                                nc.tensor.matmul(
                                    ops,
                                    lhsT=attnP[p][g][:, j * P:(j + 1) * P],
                                    rhs=wo_sb[p][:, eg * QG:(eg + 1) * QG],
                                    start=(p == 0), stop=(p == HL // 2 - 1))
                            otap = ot[:, eg * QG:(eg + 1) * QG]
                            if alt and eg == 1:
                                nc.scalar.copy(otap, ops)
                            else:
                                nc.vector.tensor_copy(otap, ops)
                            if drain:
                                # final group: per-half DMAs start right
                                # after each copy, shortening the epilogue
                                nc.sync.dma_start(
                                    out=d_out[st * P:(st + 1) * P,
                                              eg * QG:(eg + 1) * QG],
                                    in_=otap)
                            elif eg == 1:
                                nc.sync.dma_start(
                                    out=d_out[st * P:(st + 1) * P, :], in_=ot)
                        return eg_piece
                    f = mk()
                    pieces.append(lambda f=f: f(0))
                    pieces.append(lambda f=f: f(1))
                return pieces

            pending_out = []

            def sprinkle(n):
                for _ in range(n):
                    if pending_out:
                        pending_out.pop(0)()

            # head-task software pipeline: scores of task i are emitted
            # before the PV of task i-1, so every PV's exp wait is covered
            # by the next head's score matmuls; the zinv/attn chain trails
            # one more task behind.
            tasks = [(g, h) for g in g_order for h in range(HL)]
            staged = {}   # i -> plist | (pv, zrb) as it advances
            fins = {}

            def stage_probs(i):
                g, h = tasks[i]
                plist = probs_cache.pop((h, g), None)
                if plist is None:
                    plist = emit_probs(h, g)
                staged[i] = plist

            def stage_pv(i):
                g, h = tasks[i]
                fins[i] = emit_pv(h, g, staged.pop(i))

            def stage_fin(i):
                # fires on odd-head tasks only: finishes the whole pair
                g, h = tasks[i]
                if h % 2 == 0:
                    return
                emit_pairfin(g, h // 2, fins.pop(i - 1), fins.pop(i))
                if h == HL - 1:
                    pending_out.extend(
                        outproj_pieces(g, alt=(g in g_order[-2:]),
                                       drain=(g == g_order[-1])))

            nt = len(tasks)
            si = 0
            for j in range(nt):
                # scores lookahead keeps the exp chain ahead of the PV stream
                depth = 3 if j >= nt - 13 else 2
                while si < min(nt, j + depth + 1):
                    stage_probs(si)
                    si += 1
                sprinkle(1)
                stage_pv(j)
                sprinkle(1)
                if j >= 1:
                    stage_fin(j - 1)
                sprinkle(2 if j >= nt - 8 else 1)
            stage_fin(nt - 1)
            sprinkle(len(pending_out))
            
            psO.release()
            psPV.release()
            out_pool.release()
            z_pool.release()
            probs_pool.release()
            xv_pool.release()
            psS.release()

        for _rep in range(repeat):
            emit_once()

    _split_multi_waits(nc)
    return nc


# ---------------------------------------------------------------------------
# Host entry point
# ---------------------------------------------------------------------------
LAST_EXEC_NS = None
LAST_RESULT = None


def kernel(query, key, value, mask, Wq, Wk, Wv, Wo, bo):
    global LAST_EXEC_NS, LAST_RESULT
    _install_tile_drain_patch()
    from concourse.bass_utils import run_bass_kernel_spmd

    B, S, E = 2, 2048, 1024
    H, D = 16, 64
    N_CORES = 8
    BG = 2                    # batch groups
    HG = N_CORES // BG        # head groups per batch
    HL = H // HG              # heads per core
    DIM = HL * D
    P, EC2 = 128, E // 256
    WSCALE = np.float32(16.0)

    query = np.asarray(query, dtype=np.float32)
    key = np.asarray(key, dtype=np.float32)
    value = np.asarray(value, dtype=np.float32)
    mask2d = np.asarray(mask).reshape(S, S).astype(bool)
    Wq = np.asarray(Wq, dtype=np.float32)
    Wk = np.asarray(Wk, dtype=np.float32)
    Wv = np.asarray(Wv, dtype=np.float32)
    Wo = np.asarray(Wo, dtype=np.float32)
    bo = np.asarray(bo, dtype=np.float32)

    bias_idx, biases, block_live = classify_mask(mask2d, S)
    nuniq = len(biases)
    bias_stack = (np.concatenate(biases, axis=1) if nuniq
                  else np.zeros((128, 128), np.float32))

    nc = build_nc(S, E, D, HL, bias_idx, block_live, nuniq)

    scale = np.float32(1.0 / np.sqrt(D))

    # fp8 hi/lo splits of the (transposed) activation streams, per batch
    xsplit = {}
    for b in range(BG):
        for name, arr in (("q", query), ("k", key), ("v", value)):
            hi, lo = _fp8_split(arr[b].T)
            xsplit[(b, name)] = (hi, lo)

    def arrange_w(w):  # [E, n] -> [P, EC2, 2, n] matching the sbuf layout
        n = w.shape[1]
        return np.ascontiguousarray(
            w.reshape(EC2, 2, P, n).transpose(2, 0, 1, 3))

    in_maps = []
    for c in range(N_CORES):
        b, hg = c // HG, c % HG
        cols = slice(hg * DIM, (hg + 1) * DIM)
        wv_l = Wv[:, cols].reshape(E, HL, D)
        wv_aug = np.zeros((E, HL, D + 1), np.float32)
        wv_aug[:, :, :D] = wv_l
        wv_aug = wv_aug.reshape(E, HL * (D + 1))
        m = {}
        for name in ("q", "k", "v"):
            hi, lo = xsplit[(b, name)]
            m[f"x{name}8"], m[f"x{name}r"] = hi, lo
        for name, w_eff in (("q", Wq[:, cols] * scale * WSCALE),
                            ("k", Wk[:, cols] * WSCALE),
                            ("v", wv_aug * WSCALE)):
            hi, lo = _fp8_split(w_eff)
            m[f"w{name}8"] = arrange_w(hi)
            m[f"w{name}r"] = arrange_w(lo)
        m["wo"] = _bf16(Wo[cols, :])
        m["biasT"] = _bf16(bias_stack)
        in_maps.append(m)

    res = run_bass_kernel_spmd(nc, in_maps, list(range(N_CORES)))
    LAST_RESULT = res
    LAST_EXEC_NS = res.exec_time_ns or res.mean_exec_time_ns

    out = np.empty((B, S, E), np.float32)
    for b in range(BG):
        acc = res.results[b * HG]["out_p"].astype(np.float32)
        for j in range(1, HG):
            acc = acc + res.results[b * HG + j]["out_p"].astype(np.float32)
        out[b] = acc + bo[None, :]
    return out


def _bf16(a):
    import ml_dtypes
    return np.ascontiguousarray(np.asarray(a, np.float32)).astype(
        ml_dtypes.bfloat16)


def _fp8_split(a):
    """Exact-ish hi/lo split: a ~= hi + lo with both fp8e4m3 (residual is
    ~2^-8 relative)."""
    import ml_dtypes
    a = np.ascontiguousarray(np.asarray(a, np.float32))
    hi = a.astype(ml_dtypes.float8_e4m3)
    lo = (a - hi.astype(np.float32)).astype(ml_dtypes.float8_e4m3)
    return hi, lo


# revision 84
# speedup vs baseline: 1.0054x; 1.0054x over previous
"""Multi-head attention (B=2, S=2048, E=1024, H=16) on 8 Trainium2 NeuronCores.

Sharding: core c -> batch c//4, heads 4*(c%4)..4*(c%4)+3 (data + head
parallel).  Each core emits a bf16 partial output projection [S, E] over its
256 head-dims; the host sums the 4 partials per batch in f32 and adds the
output bias (the "all-reduce" happens in the unshard step).

Numerics (validated off-line against a float64 reference and on-device:
rel err 0.0055 vs the 2e-2 gate; plain-fp8 operands measured 0.025-0.045
and are NOT used):
  * Projections run on the PE in fp8e4m3 DoubleRow mode (0.5 cycles/row,
    2 contraction chunks per instruction) using an exact hi/lo split:
    x @ W ~= x8@W8 + x8@Wr + xr@W8  (xr/Wr are the fp8 residuals; the
    dropped xr@Wr term is ~2^-8 relative).  Host precomputes all splits.
    Weights carry a x16 scale (and Wq the 1/sqrt(D) fold): QT/KT hold 16q,
    2q*... the exp activation's scale=1/256 de-scales the score product, and
    the x16 on V cancels against the x16 ones-column through the softmax
    normalization, so no extra de-scale ops exist anywhere.
  * Scores/PV/out-projection stay bf16 (DoubleRow needs both operands fp8;
    measured fp8 probs/V/attn all bust the error budget).  Score AND PV
    matmuls are column-trimmed to each diagonal key-tile's live span (the
    first PV matmul keeps full width with start=True so every PSUM address
    is initialized; the skipped columns were exact zero probs).
  * Softmax: constant-shift exp (shift=2), masking applied post-exp as a 0/1
    multiply on the diagonal blocks; Z comes from a 16-valued ones column in
    V via the same PV matmuls; zinv = DVE reciprocal -> bf16, broadcast
    across partitions with a K=1 PE matmul.

Schedule (cost-model-driven; container has no NTFF/neuron-profile path;
TimelineSim 136165 ns vs the 178675 ns bf16 baseline):
  * DMA priority order: Q then K streams (hi then lo, weight tile leading
    each), V last; the Q projection (both m-tiles, 3 split passes, e-pair
    outer, per-group PSUM copies emitted the moment each group stops) runs
    while its chunks stream, then K.  Scores for 3 heads of the first
    q-group are emitted as a prelude that fills the xv-stream window (one
    more head mid-V); the V projection is two-stage (hi passes -> f32 acc
    in SBUF, residual pass + DVE add) so its PSUM groups never wait on the
    lo stream.
  * Attention runs over a flattened (group, head) task list -- group order
    [largest, 2nd-smallest, 2nd-largest, smallest]: the prelude then hides
    the largest group's exp chain under the V phase, and the kernel tail
    ends on the smallest group --
    software-pipelined: scores(i+1..i+depth) are emitted before PV(i) so
    every PV's exp wait is covered by the next head's matmuls, and the
    zinv/attn chain (DVE reciprocals staging a head-pair's zinv rows at
    partitions 0/D of a zeroed tile -> one K=D+1 selector matmul broadcasts
    both -> DVE muls of the SBUF ev copies with the PSUM broadcast;
    hardware allows only one PSUM operand per vector op) trails a pair
    behind.  The staging tile MUST be memset first: its zero-weight rows
    still enter the contraction and uninitialized SBUF NaNs poison it on
    real hardware (the simulator runs no numerics).
  * The previous group's output projection is sprinkled one PSUM-group at
    a time between the scores/PV/fin stages so the 2-deep psO ring (shared
    with the zinv broadcast) never stalls the PE; the last two groups'
    second e-half copies go to ACT to unload the DVE in the tail, and the
    final group's output DMAs go per e-half so the epilogue's last transfer
    starts right after its copy instead of behind the whole-stile chain.
    The last two groups' ev copies also run on ACT: in the tail the DVE
    queue (muls + out-copies) is the local bottleneck while exp is small.
PSUM budget: psS 2x[P,2,QG] (4 banks) + psPV 2x[D+1,QG] (2) + psO/bcast
2x[P,QG] (2) = 8 banks exactly.
"""

import sys

for _p in ("/opt/trn_rl_repo", "/root/.axon_site/_ro/trn_rl_repo"):
    if _p not in sys.path:
        sys.path.insert(0, _p)

import numpy as np


# ---------------------------------------------------------------------------
# Patch: the walrus build in this container rejects >1 sem wait on one CTRL
# instruction ("Too many sync wait commands") and the TileContext exit drain
# aggregates every outstanding proc's wait onto a single Drain. Spill the
# excess waits onto SP nops (1 wait each) emitted right after the drain.
# ---------------------------------------------------------------------------
def _install_tile_drain_patch():
    import concourse.tile as tile
    import concourse.mybir as mybir
    from concourse.vector_clock import ScopedClock

    if getattr(tile.TileContext, "_drain_patch_installed", False):
        return

    def _patched_drain_and_barrier(self, tick_clock, wait_clock):
        drain_inst = self.nc.sync.drain()
        wait_clock.add_sem_waits(
            drain_inst.ins, ScopedClock({None: tick_clock.global_clock})
        )
        si = drain_inst.ins.sync_info
        waits = list(si.on_wait) if si and si.on_wait else []
        if len(waits) > 1:
            si.on_wait = waits[:1]
            for w in waits[1:]:
                nop = self.nc.sync.nop(nofuse=True, hint="drain_wait_spill")
                nop.ins.sync_info = mybir.SyncInfo(on_wait=[w], on_update=[])
        self.nc.all_engine_barrier()
        assert self.sems is not None
        popped = self.nc._tile_sem_poison_stack.pop()
        assert popped is self._sem_poison
        self.nc.clear_and_free_semaphores(list(self.sems.allocated().values()))
        self.nc.all_engine_barrier()

    tile.TileContext._drain_and_barrier = _patched_drain_and_barrier
    tile.TileContext._drain_patch_installed = True


def _split_multi_waits(nc, maxw=1):
    """Walrus here allows only `maxw` sem-wait commands per instruction.
    Hoist excess waits onto engine-queue NoOps inserted just before the
    instruction (the sequencer executes them in order, so semantics are
    identical)."""
    import concourse.mybir as mybir

    ctr = 0
    for bb in nc.main_func.blocks:
        new = []
        for inst in bb.instructions:
            si = inst.sync_info
            waits = list(si.on_wait) if si and si.on_wait else []
            if len(waits) > maxw:
                extras = waits[:-maxw]
                si.on_wait = waits[-maxw:]
                for i in range(0, len(extras), maxw):
                    nop = mybir.InstNoOp(
                        name=f"I-waitspill-{ctr}", engine=inst.engine,
                        ins=[], outs=[])
                    ctr += 1
                    nop.sync_info = mybir.SyncInfo(
                        on_wait=extras[i:i + maxw], on_update=[])
                    try:
                        nc.register_instruction(nop, overwrite=True)
                    except Exception:
                        pass
                    new.append(nop)
            new.append(inst)
        bb.instructions = new


# ---------------------------------------------------------------------------
# Mask classification (host side, from the actual mask array).
# Blocks are 128x128 in the *transposed* score layout: block (kt, qb) covers
# keys kt*128.. x queries qb*128... Returns per-block bias indices into a
# stack of unique multiplicative 0/1 mask blocks.
# ---------------------------------------------------------------------------
def classify_mask(mask2d, S, KB=128):
    nb = S // KB
    assert mask2d.shape == (S, S)
    assert mask2d.any(axis=1).all(), "a query row with no attended key"
    maskT = mask2d.T  # [keys, q]
    uniq = {}
    biases = []
    bias_idx = {}  # (kt, qb) -> None (all attended) or index
    block_live = np.zeros((nb, nb), dtype=bool)  # any attended key in block
    for kt in range(nb):
        for qb in range(nb):
            blk = maskT[kt * KB:(kt + 1) * KB, qb * KB:(qb + 1) * KB]
            if blk.all():
                bias_idx[(kt, qb)] = None
                block_live[kt, qb] = True
            else:
                b = np.where(blk, np.float32(1.0), np.float32(0.0))
                key = b.tobytes()
                if key not in uniq:
                    uniq[key] = len(biases)
                    biases.append(b)
                bias_idx[(kt, qb)] = uniq[key]
                block_live[kt, qb] = blk.any()
    return bias_idx, biases, block_live


# ---------------------------------------------------------------------------
# Bass program builder (one SPMD program, same for all cores).
# ---------------------------------------------------------------------------
def build_nc(S, E, D, HL, bias_idx, block_live, nuniq, shift=2.0, repeat=1):
    import concourse.bass as bass
    import concourse.mybir as mybir
    import concourse.tile as tile

    f32 = mybir.dt.float32
    bf16 = mybir.dt.bfloat16
    f8 = mybir.dt.float8e4
    Act = mybir.ActivationFunctionType
    DR = mybir.MatmulPerfMode.DoubleRow
    ADD = mybir.AluOpType.add

    P = 128
    EC2 = E // 256           # contraction chunk-pairs (4)
    DIM = HL * D             # this core's head dims (256)
    MT = DIM // P            # m-tiles of QT/KT (2)
    QG = 512                 # q-group width
    NQG = S // QG
    QB = QG // P             # q-blocks per group
    NKT = S // P             # key tiles
    NST = S // P
    VW = HL * (D + 1)        # V width incl. ones columns (260)
    SCINV = 1.0 / 256.0      # undo the 2x/16x weight scales in the exp
    VONES = 16.0             # ones-column value matching the 16x on Wv
    N_PRELUDE = 3            # heads of the first q-group emitted early

    def kts_for_group(g):
        out = []
        for kt in range(NKT):
            if any(block_live[kt, g * QB + j] for j in range(QB)):
                out.append(kt)
        return out

    def span_start(kt, g):
        js = [j for j in range(QB) if block_live[kt, g * QB + j]]
        return min(js) * P

    nc = bass.Bass()
    dp = nc.declare_dram_parameter
    d_x = {}
    for t in ("q", "k", "v"):
        for sfx in ("8", "r"):
            d_x[t + sfx] = dp(f"x{t}{sfx}", [E, S], f8, isOutput=False)
    WWID = {"q": DIM, "k": DIM, "v": VW}
    d_w = {}
    for t, wid in WWID.items():
        for sfx in ("8", "r"):
            d_w[t + sfx] = dp(f"w{t}{sfx}", [P, EC2, 2, wid], f8,
                              isOutput=False)
    d_wo = dp("wo", [DIM, E], bf16, isOutput=False)
    d_bias = dp("biasT", [P, max(nuniq, 1) * P], bf16, isOutput=False)
    d_out = dp("out_p", [S, E], bf16, isOutput=True)

    import contextlib
    with tile.TileContext(nc) as tc, contextlib.ExitStack() as _stk:
        consts = _stk.enter_context(tc.tile_pool(name="consts", bufs=1))

        w_sb = {}
        for t, wid in WWID.items():
            for sfx in ("8", "r"):
                k = t + sfx
                w_sb[k] = consts.tile([P, EC2, 2, wid], f8, name=f"sb_w{k}",
                                      tag=f"sb_w{k}")
        wo_sb = [consts.tile([2 * D, E], bf16, name=f"sb_wo{p}",
                             tag=f"sb_wo{p}") for p in range(HL // 2)]
        bias_sb = consts.tile([P, max(nuniq, 1) * P], bf16, name="sb_bias")
        ones128 = consts.tile([P, D], bf16, name="ones128")
        nc.vector.memset(ones128, 1.0)
        # pair-broadcast selector: routes z2 row 0 -> out rows 0..D-1 and
        # z2 row D -> out rows D..2D-1 in a single K=D+1 matmul
        ones2 = consts.tile([D + 1, 2 * D], bf16, name="ones2")
        nc.vector.memset(ones2, 0.0)
        nc.vector.memset(ones2[0:1, 0:D], 1.0)
        nc.vector.memset(ones2[D:D + 1, D:2 * D], 1.0)
        negshift = consts.tile([P, 1], f32, name="negshift")
        nc.vector.memset(negshift, -shift)

        def emit_once():
            QT = [consts.tile([P, S], bf16, name=f"QT{m}", tag=f"QT{m}")
                  for m in range(MT)]
            KT = [consts.tile([P, S], bf16, name=f"KT{m}", tag=f"KT{m}")
                  for m in range(MT)]
            V = [consts.tile([P, VW], bf16, name=f"V{s}", tag=f"V{s}")
                 for s in range(NST)]
            attnP = [[consts.tile([2 * D, QG], bf16, name=f"attnP{p}g{g}",
                                  tag=f"attnP{p}g{g}") for g in range(NQG)]
                     for p in range(HL // 2)]

            # ---------------- DMA emission (order = priority) -------------
            # pool alloc/release must be LIFO per memory space
            xv_pool = tc.alloc_tile_pool(name="xvp", bufs=1)
            xqk_pool = tc.alloc_tile_pool(name="xqk", bufs=1)

            x_sb = {}

            def load_x(pool, key, ep, halves=1):
                t = pool.tile([P, 2, S], f8, name=f"x{key}e{ep}",
                              tag=f"x{key}e{ep}")
                src_ap = d_x[key][ep * 256:(ep + 1) * 256, :].rearrange(
                    "(t p) s -> p t s", p=P)
                hw = S // halves
                for i in range(halves):
                    nc.sync.dma_start(out=t[:, :, i * hw:(i + 1) * hw],
                                      in_=src_ap[:, :, i * hw:(i + 1) * hw])
                x_sb.setdefault(key, {})[ep] = t

            def load_w(key, split=False):
                if split:
                    # per-chunk-pair slices so the first projection matmul
                    # only waits for a 64KB transfer, not the whole tile
                    for ep in range(EC2):
                        nc.sync.dma_start(out=w_sb[key][:, ep, :, :],
                                          in_=d_w[key][:, ep, :, :])
                else:
                    nc.sync.dma_start(out=w_sb[key], in_=d_w[key][:, :, :, :])

            # each stream leads with its (small) weight tile so the first
            # chunk's matmul has both operands as early as possible
            for key, pool in (("q8", xqk_pool), ("qr", xqk_pool),
                              ("k8", xqk_pool), ("kr", xqk_pool),
                              ("v8", xv_pool), ("vr", xv_pool)):
                load_w(key)
                for ep in range(EC2):
                    load_x(pool, key, ep)
            for p in range(HL // 2):
                nc.sync.dma_start(
                    out=wo_sb[p], in_=d_wo[p * 2 * D:(p + 1) * 2 * D, :])
            nc.sync.dma_start(out=bias_sb, in_=d_bias[:, :])

            PASSES = (("8", "8"), ("8", "r"), ("r", "8"))

            # ---------------- Q then K projections (fp8 split) ------------
            psQK = tc.alloc_tile_pool(name="psQK", bufs=1, space="PSUM")

            def emit_proj(proj, dst, copy_eng):
                ps = {}
                for m in range(MT):
                    for g in range(NQG):
                        ps[m, g] = psQK.tile([P, QG], f32,
                                             tag=f"psqk{m}{g}",
                                             name=f"ps{proj}{m}{g}")
                def copy_out(m, g):
                    # engines split per m-tile so the m0 tiles (which gate
                    # the prelude scores) drain in parallel with m1
                    eng = copy_eng[m]
                    dstap = dst[m][:, g * QG:(g + 1) * QG]
                    if eng == "act":
                        nc.scalar.copy(dstap, ps[m, g])
                    elif eng == "pool":
                        nc.gpsimd.tensor_copy(dstap, ps[m, g])
                    else:
                        nc.vector.tensor_copy(dstap, ps[m, g])

                for pi, (xs, ws) in enumerate(PASSES):
                    for ep in range(EC2):
                        last = pi == 2 and ep == EC2 - 1
                        for m in range(MT):
                            for g in range(NQG):
                                nc.tensor.matmul(
                                    ps[m, g],
                                    lhsT=w_sb[proj + ws][
                                        :, ep, :, m * P:(m + 1) * P],
                                    rhs=x_sb[proj + xs][ep][
                                        :, :, g * QG:(g + 1) * QG],
                                    start=(pi == 0 and ep == 0),
                                    stop=last,
                                    perf_mode=DR)
                                if last:
                                    # copy each group the moment it stops, so
                                    # the first scores don't wait for all 8
                                    copy_out(m, g)

            emit_proj("q", QT, {0: "dve", 1: "dve"})
            emit_proj("k", KT, {0: "act", 1: "dve"})
            psQK.release()
            xqk_pool.release()

            # ---------------- phase-B pools -------------------------------
            psS = tc.alloc_tile_pool(name="psS", bufs=2, space="PSUM")
            probs_pool = tc.alloc_tile_pool(name="probs", bufs=34)
            z_pool = tc.alloc_tile_pool(name="zrb", bufs=4)
            out_pool = tc.alloc_tile_pool(name="outst", bufs=4)

            def emit_probs(h, g):
                m, po = h // 2, (h % 2) * D
                kts = kts_for_group(g)
                pairs = [kts[i:i + 2] for i in range(0, len(kts), 2)]
                plist = []
                for pair in pairs:
                    sps = psS.tile([P, 2, QG], f32, tag="s")
                    pb = probs_pool.tile([P, 2, QG], bf16, tag="pb")
                    spans = [span_start(kt, g) for kt in pair]
                    for x, kt in enumerate(pair):
                        s0 = spans[x]
                        nc.tensor.matmul(
                            sps[:, x, s0:],
                            lhsT=KT[m][po:po + D, kt * P:(kt + 1) * P],
                            rhs=QT[m][po:po + D, g * QG + s0:(g + 1) * QG],
                            start=True, stop=True)
                    if len(pair) == 2:
                        # one op from the smaller span: any dead columns it
                        # covers are re-zeroed by the masking memsets below
                        s0 = min(spans)
                        nc.scalar.activation(
                            pb[:, :, s0:], sps[:, :, s0:],
                            Act.Exp, bias=negshift[:, 0:1], scale=SCINV)
                    else:
                        nc.scalar.activation(
                            pb[:, 0, spans[0]:], sps[:, 0, spans[0]:],
                            Act.Exp, bias=negshift[:, 0:1], scale=SCINV)
                    for x, kt in enumerate(pair):
                        s0 = spans[x]
                        for j in range(QB):
                            qb = g * QB + j
                            if j * P < s0:
                                continue
                            bidx = bias_idx[(kt, qb)]
                            if bidx is None:
                                continue
                            blk = pb[:, x, j * P:(j + 1) * P]
                            if not block_live[kt, qb]:
                                nc.gpsimd.memset(blk, 0.0)
                            else:
                                nc.vector.tensor_mul(
                                    blk, blk,
                                    bias_sb[:, bidx * P:(bidx + 1) * P])
                    plist.append((pair, pb, spans))
                return plist

            by_size = sorted(range(NQG), key=lambda g: -len(kts_for_group(g)))
            g_order = ([by_size[0], by_size[2], by_size[1], by_size[3]]
                       if NQG > 3 else by_size)

            # prelude: scores for the first heads of the largest q-group fill
            # the xv-stream window
            probs_cache = {}
            for h in range(N_PRELUDE):
                probs_cache[(h, g_order[0])] = emit_probs(h, g_order[0])

            # ---------------- V projection (two-stage fp8 split) ----------
            psV = tc.alloc_tile_pool(name="psV", bufs=1, space="PSUM")
            accv_pool = tc.alloc_tile_pool(name="accv", bufs=1)
            accV = [accv_pool.tile([P, VW], f32, tag=f"accv{st}",
                                   name=f"accv{st}") for st in range(NST)]
            for quarter in range(NST // 4):
                sts = range(quarter * 4, quarter * 4 + 4)
                pss = [psV.tile([P, VW], f32, tag=f"psv{i}", name=f"psv{i}")
                       for i in range(4)]
                for pi, (xs, ws) in enumerate(PASSES[:2]):
                    for ep in range(EC2):
                        for i, st in enumerate(sts):
                            nc.tensor.matmul(
                                pss[i],
                                lhsT=x_sb["v" + xs][ep][
                                    :, :, st * P:(st + 1) * P],
                                rhs=w_sb["v" + ws][:, ep, :, :],
                                start=(pi == 0 and ep == 0),
                                stop=(pi == 1 and ep == EC2 - 1),
                                perf_mode=DR)
                for i, st in enumerate(sts):
                    nc.vector.tensor_copy(accV[st], pss[i])
                if quarter == 0 and N_PRELUDE < HL:
                    # one more head of early scores fills the xv-stream wait
                    probs_cache[(N_PRELUDE, g_order[0])] = emit_probs(
                        N_PRELUDE, g_order[0])
            for quarter in range(NST // 4):
                sts = range(quarter * 4, quarter * 4 + 4)
                pss = [psV.tile([P, VW], f32, tag=f"psv{i}", name=f"psvr{i}")
                       for i in range(4)]
                for ep in range(EC2):
                    for i, st in enumerate(sts):
                        nc.tensor.matmul(
                            pss[i],
                            lhsT=x_sb["vr"][ep][:, :, st * P:(st + 1) * P],
                            rhs=w_sb["v8"][:, ep, :, :],
                            start=(ep == 0), stop=(ep == EC2 - 1),
                            perf_mode=DR)
                for i, st in enumerate(sts):
                    nc.vector.tensor_tensor(V[st], pss[i], accV[st], ADD)
                    onescols = V[st].rearrange(
                        "p (h c) -> p h c", c=D + 1)[:, :, D]
                    nc.gpsimd.memset(onescols, VONES)
            accv_pool.release()
            psV.release()

            # ---------------- attention main loop -------------------------
            psPV = tc.alloc_tile_pool(name="psPV", bufs=2, space="PSUM")
            # the zinv-broadcast psum and the out-projection psum share one
            # 2-deep [P, QG] ring: the broadcast only borrows it 16 times,
            # and outproj pieces get double-buffering so the PE never waits
            # for the previous piece's PSUM->SBUF copy
            psO = tc.alloc_tile_pool(name="psO", bufs=2, space="PSUM")

            def emit_pv(h, g, plist):
                kts = kts_for_group(g)
                pv = psPV.tile([D + 1, QG], f32, tag="pv")
                n = 0
                for pair, pb, spans in plist:
                    for x, kt in enumerate(pair):
                        s0 = 0 if n == 0 else spans[x]
                        nc.tensor.matmul(
                            pv[:, s0:],
                            lhsT=V[kt][:, h * (D + 1):(h + 1) * (D + 1)],
                            rhs=pb[:, x, s0:],
                            start=(n == 0), stop=(n == len(kts) - 1))
                        n += 1
                # zinv rows of the two heads of a pair stage at partitions
                # 0 and D of a shared tile for a single pair-broadcast matmul
                if h % 2 == 0:
                    z2 = z_pool.tile([D + 1, QG], bf16, tag="z2")
                    # the unwritten rows take part in the selector
                    # contraction with zero weights; clear them so they
                    # cannot hold NaN/Inf garbage (the recips then overwrite
                    # rows 0 and D)
                    nc.gpsimd.memset(z2, 0.0)
                    pair_z2[(g, h // 2)] = z2
                else:
                    z2 = pair_z2[(g, h // 2)]
                with nc.allow_low_precision("zinv broadcast is bf16, as was "
                                            "the baseline's exp(-ln Z)"):
                    nc.vector.reciprocal(z2[(h % 2) * D:(h % 2) * D + 1, :],
                                         pv[D:D + 1, :])
                ev = z_pool.tile([D, QG], f32, tag="ev")
                if g in g_order[-2:]:
                    # tail groups: the DVE queue is locally saturated with
                    # muls and out-copies while exp work is small -> ACT
                    nc.scalar.copy(ev, pv[0:D, :])
                else:
                    nc.vector.tensor_copy(ev, pv[0:D, :])
                return ev

            pair_z2 = {}

            def emit_pairfin(g, p, ev0, ev1):
                # one K=D+1 matmul broadcasts both heads' zinv rows across
                # their 64-partition halves; the muls pair the SBUF ev
                # copies with the PSUM broadcast (hardware allows only one
                # PSUM operand per vector op)
                z2 = pair_z2.pop((g, p))
                obt = psO.tile([P, QG], f32, tag="o")
                nc.tensor.matmul(obt, lhsT=ones2[:, :], rhs=z2[:, :],
                                 start=True, stop=True)
                nc.vector.tensor_mul(attnP[p][g][0:D, :], ev0, obt[0:D, :])
                nc.vector.tensor_mul(attnP[p][g][D:2 * D, :], ev1,
                                     obt[D:2 * D, :])

            def outproj_pieces(g, alt=False, drain=False):
                # one closure per (stile, e-half): a single psO group each,
                # sprinkled into the scores stream so the 2-deep psO ring
                # never stalls the PE; the PSUM->SBUF bounce is on the DVE
                # (GPSIMD cannot read PSUM on hardware)
                pieces = []
                for j in range(QB):
                    st = g * QB + j
                    ot = out_pool.tile([P, E], bf16, tag="ot")

                    def mk(j=j, st=st, ot=ot):
                        def eg_piece(eg):
                            ops = psO.tile([P, QG], f32, tag="o")
                            for p in range(HL // 2):
                                nc.tensor.matmul(
                                    ops,
                                    lhsT=attnP[p][g][:, j * P:(j + 1) * P],
                                    rhs=wo_sb[p][:, eg * QG:(eg + 1) * QG],
                                    start=(p == 0), stop=(p == HL // 2 - 1))
                            otap = ot[:, eg * QG:(eg + 1) * QG]
                            if alt and eg == 1:
                                nc.scalar.copy(otap, ops)
                            else:
                                nc.vector.tensor_copy(otap, ops)
                            if drain:
                                # final group: per-half DMAs start right
                                # after each copy, shortening the epilogue
                                nc.sync.dma_start(
                                    out=d_out[st * P:(st + 1) * P,
                                              eg * QG:(eg + 1) * QG],
                                    in_=otap)
                            elif eg == 1:
                                nc.sync.dma_start(
                                    out=d_out[st * P:(st + 1) * P, :], in_=ot)
                        return eg_piece
                    f = mk()
                    pieces.append(lambda f=f: f(0))
                    pieces.append(lambda f=f: f(1))
                return pieces

            pending_out = []

            def sprinkle(n):
                for _ in range(n):
                    if pending_out:
                        pending_out.pop(0)()

            # head-task software pipeline: scores of task i are emitted
            # before the PV of task i-1, so every PV's exp wait is covered
            # by the next head's score matmuls; the zinv/attn chain trails
            # one more task behind.
            tasks = [(g, h) for g in g_order for h in range(HL)]
            staged = {}   # i -> plist | (pv, zrb) as it advances
            fins = {}

            def stage_probs(i):
                g, h = tasks[i]
                plist = probs_cache.pop((h, g), None)
                if plist is None:
                    plist = emit_probs(h, g)
                staged[i] = plist

            def stage_pv(i):
                g, h = tasks[i]
                fins[i] = emit_pv(h, g, staged.pop(i))

            def stage_fin(i):
                # fires on odd-head tasks only: finishes the whole pair
                g, h = tasks[i]
                if h % 2 == 0:
                    return
                emit_pairfin(g, h // 2, fins.pop(i - 1), fins.pop(i))
                if h == HL - 1:
                    pending_out.extend(
                        outproj_pieces(g, alt=(g in g_order[-2:]),
                                       drain=(g == g_order[-1])))

            nt = len(tasks)
            si = 0
            for j in range(nt):
                # scores lookahead keeps the exp chain ahead of the PV stream
                depth = 3
                while si < min(nt, j + depth + 1):
                    stage_probs(si)
                    si += 1
                sprinkle(1)
                stage_pv(j)
                sprinkle(1)
                if j >= 1:
                    stage_fin(j - 1)
                sprinkle(2 if j >= nt - 8 else 1)
            stage_fin(nt - 1)
            sprinkle(len(pending_out))
            
            psO.release()
            psPV.release()
            out_pool.release()
            z_pool.release()
            probs_pool.release()
            xv_pool.release()
            psS.release()

        for _rep in range(repeat):
            emit_once()

    _split_multi_waits(nc)
    return nc


# ---------------------------------------------------------------------------
# Host entry point
# ---------------------------------------------------------------------------
LAST_EXEC_NS = None
LAST_RESULT = None


def kernel(query, key, value, mask, Wq, Wk, Wv, Wo, bo):
    global LAST_EXEC_NS, LAST_RESULT
    _install_tile_drain_patch()
    from concourse.bass_utils import run_bass_kernel_spmd

    B, S, E = 2, 2048, 1024
    H, D = 16, 64
    N_CORES = 8
    BG = 2                    # batch groups
    HG = N_CORES // BG        # head groups per batch
    HL = H // HG              # heads per core
    DIM = HL * D
    P, EC2 = 128, E // 256
    WSCALE = np.float32(16.0)

    query = np.asarray(query, dtype=np.float32)
    key = np.asarray(key, dtype=np.float32)
    value = np.asarray(value, dtype=np.float32)
    mask2d = np.asarray(mask).reshape(S, S).astype(bool)
    Wq = np.asarray(Wq, dtype=np.float32)
    Wk = np.asarray(Wk, dtype=np.float32)
    Wv = np.asarray(Wv, dtype=np.float32)
    Wo = np.asarray(Wo, dtype=np.float32)
    bo = np.asarray(bo, dtype=np.float32)

    bias_idx, biases, block_live = classify_mask(mask2d, S)
    nuniq = len(biases)
    bias_stack = (np.concatenate(biases, axis=1) if nuniq
                  else np.zeros((128, 128), np.float32))

    nc = build_nc(S, E, D, HL, bias_idx, block_live, nuniq)

    scale = np.float32(1.0 / np.sqrt(D))

    # fp8 hi/lo splits of the (transposed) activation streams, per batch
    xsplit = {}
    for b in range(BG):
        for name, arr in (("q", query), ("k", key), ("v", value)):
            hi, lo = _fp8_split(arr[b].T)
            xsplit[(b, name)] = (hi, lo)

    def arrange_w(w):  # [E, n] -> [P, EC2, 2, n] matching the sbuf layout
        n = w.shape[1]
        return np.ascontiguousarray(
            w.reshape(EC2, 2, P, n).transpose(2, 0, 1, 3))

    in_maps = []
    for c in range(N_CORES):
        b, hg = c // HG, c % HG
        cols = slice(hg * DIM, (hg + 1) * DIM)
        wv_l = Wv[:, cols].reshape(E, HL, D)
        wv_aug = np.zeros((E, HL, D + 1), np.float32)
        wv_aug[:, :, :D] = wv_l
        wv_aug = wv_aug.reshape(E, HL * (D + 1))
        m = {}
        for name in ("q", "k", "v"):
            hi, lo = xsplit[(b, name)]
            m[f"x{name}8"], m[f"x{name}r"] = hi, lo
        for name, w_eff in (("q", Wq[:, cols] * scale * WSCALE),
                            ("k", Wk[:, cols] * WSCALE),
                            ("v", wv_aug * WSCALE)):
            hi, lo = _fp8_split(w_eff)
            m[f"w{name}8"] = arrange_w(hi)
            m[f"w{name}r"] = arrange_w(lo)
        m["wo"] = _bf16(Wo[cols, :])
        m["biasT"] = _bf16(bias_stack)
        in_maps.append(m)

    res = run_bass_kernel_spmd(nc, in_maps, list(range(N_CORES)))
    LAST_RESULT = res
    LAST_EXEC_NS = res.exec_time_ns or res.mean_exec_time_ns

    out = np.empty((B, S, E), np.float32)
    for b in range(BG):
        acc = res.results[b * HG]["out_p"].astype(np.float32)
        for j in range(1, HG):
            acc = acc + res.results[b * HG + j]["out_p"].astype(np.float32)
        out[b] = acc + bo[None, :]
    return out


def _bf16(a):
    import ml_dtypes
    return np.ascontiguousarray(np.asarray(a, np.float32)).astype(
        ml_dtypes.bfloat16)


def _fp8_split(a):
    """Exact-ish hi/lo split: a ~= hi + lo with both fp8e4m3 (residual is
    ~2^-8 relative)."""
    import ml_dtypes
    a = np.ascontiguousarray(np.asarray(a, np.float32))
    hi = a.astype(ml_dtypes.float8_e4m3)
    lo = (a - hi.astype(np.float32)).astype(ml_dtypes.float8_e4m3)
    return hi, lo
